# revision 9
# baseline (speedup 1.0000x reference)
"""GCN (3-layer GraphConv, norm='both') on 8 Trainium2 NeuronCores.

Self-contained: takes FULL inputs, returns FULL output [N, n_classes].

Strategy
--------
Math: per layer  out = nd * (A^T (ns * (x @ W))) + b,  nd=deg_in^-1/2, ns=deg_out^-1/2
(W pushed in front of the SpMM by associativity; for layer 3 this shrinks the
gather width from 128 to 40->64 floats).

- Nodes are sorted by in-degree (desc) and chopped into 128-row tiles; tiles are
  dealt round-robin to the 8 cores so every core's j-th tile has near-identical
  degree profile (SPMD: one NEFF, per-core index data).
- Per layer, each core computes the table rows Z = ns*(H@W) for its own nodes
  (node-major [6272, d]) and an AllGather materializes the full table
  [50176, d] in every core's DRAM (addr_space="Shared").
- SpMM = ELL gather-accumulate: for each dst tile, dma_gather pulls the source
  rows (one row per (slot, step)), then a log-tree of wide DVE adds reduces the
  steps. dma_gather indices are int16, so the table is addressed through two
  overlapping views A=[0,32768) and B=[NTOT-32768,NTOT); each node's in-edges
  are split between the views (flexible middle zone balances the split).
  Padding slots gather a guaranteed-zero table row (virtual node rows).
- relu(nd*agg)*ns == relu(nd*ns*agg) for zero bias -> single fused dual-op
  tensor_scalar per tile. (Non-zero bias gets an explicit slower path.)
- Next-layer table tile: PE transpose (identity) -> matmul with W -> DMA out.
"""

import math
import os

import numpy as np

P = 128
NCORES = 8
MAXK = int(os.environ.get("GCN_MAXK", "8"))  # max gather steps per dma_gather


# ----------------------------------------------------------------------------
# Host-side preprocessing
# ----------------------------------------------------------------------------

def _preprocess(features, edge_index, W1, b1, W2, b2, W3, b3):
    n, din = features.shape
    dhid = W2.shape[0]
    ncls = W3.shape[1]
    assert din == P and dhid == P, "kernel assumes 128-wide features"
    dcp = ((ncls + 63) // 64) * 64  # pad classes to 64 (256B rows for dma_gather)

    src = np.asarray(edge_index[0], dtype=np.int64)
    dst = np.asarray(edge_index[1], dtype=np.int64)

    deg_out = np.bincount(src, minlength=n).astype(np.float32)
    deg_in = np.bincount(dst, minlength=n).astype(np.float32)
    ns = np.maximum(deg_out, 1.0) ** -0.5
    nd = np.maximum(deg_in, 1.0) ** -0.5

    # --- layout sizes; table views (int16 index limit 32767)
    tpc = math.ceil((n + 1) / (P * NCORES))       # tiles per core
    s0 = tpc * P                                   # slots per core
    ntot = NCORES * s0
    nv = ntot - n                                  # virtual (zero) rows
    assert nv >= 1
    HIGH = min(32768, ntot)   # view A = [0, HIGH)
    LOW = ntot - HIGH         # view B = [LOW, ntot)

    # Table zones decide edge forcing: sources with table id in
    #   [0, LOW)      -> only reachable via view A  ("forced A")
    #   [LOW, HIGH)   -> reachable via both views    ("flex")
    #   [HIGH, ntot)  -> only view B                 ("forced B")
    zone_lo = np.array([0, LOW, HIGH])
    zone_hi = np.array([LOW, HIGH, ntot])
    zone_size = zone_hi - zone_lo
    # reserved virtual table ids: 0 (view-A zero row) and the tail
    # [ntot-nv+1, ntot) (so id ntot-1 is the view-B zero row)
    resv = np.zeros(3, dtype=np.int64)
    z_of_id0 = 0 if LOW > 0 else 1
    resv[z_of_id0] += 1
    tail_lo = ntot - (nv - 1)
    for z in range(3):
        resv[z] += max(0, min(zone_hi[z], ntot) - max(zone_lo[z], tail_lo))
    real_cap = zone_size - resv
    assert real_cap.sum() == n and (real_cap >= 0).all()

    # --- assign real nodes to zones: degree-sorted, dealt proportionally so
    #     every zone sees the same degree profile
    order = np.argsort(-deg_in, kind="stable")
    zone_of_old = np.empty(n, dtype=np.int8)
    cnt = np.zeros(3, dtype=np.int64)
    err = np.zeros(3)
    frac = real_cap / max(n, 1)
    for idx in order:
        err += frac
        z = int(np.argmax(np.where(cnt < real_cap, err, -np.inf)))
        err[z] -= 1.0
        cnt[z] += 1
        zone_of_old[idx] = z

    # --- forcing + provisional balanced split per dst node
    sz = zone_of_old[src]
    forced_a = sz == 0
    forced_b = sz == 2
    flex = sz == 1
    n_a = np.bincount(dst[forced_a], minlength=n)
    n_b = np.bincount(dst[forced_b], minlength=n)
    n_f = np.bincount(dst[flex], minlength=n)
    deg = n_a + n_b + n_f
    x_prov = np.clip((n_b - n_a + n_f + 1) // 2, 0, n_f)
    ka_prov = n_a + x_prov
    kb_prov = n_b + (n_f - x_prov)

    # --- group profile-similar nodes into tiles; place tiles so that the
    #     8 cores' j-th tiles have matching profiles (SPMD-uniform K)
    queues = []
    for z in range(3):
        ids_z = np.nonzero(zone_of_old == z)[0]
        o = np.lexsort((-deg[ids_z], -kb_prov[ids_z], -ka_prov[ids_z]))
        q = ids_z[o].tolist()
        if z == z_of_id0:
            q = [-1] + q
        # tail virtuals for this zone
        extra = int(resv[z] - (1 if z == z_of_id0 else 0))
        q = q + [-1] * extra
        queues.append(q)
    cur = [0, 0, 0]
    old_of_new = np.full(ntot, -1, dtype=np.int64)
    for j in range(tpc):
        for c in range(NCORES):
            base = c * s0 + j * P
            z = 0 if base < LOW else (1 if base < HIGH else 2)
            chunk = queues[z][cur[z]:cur[z] + P]
            cur[z] += P
            old_of_new[base:base + P] = chunk
    assert all(cur[z] == len(queues[z]) for z in range(3))
    assert old_of_new[0] == -1 and old_of_new[ntot - 1] == -1
    mask_real = old_of_new >= 0
    new_of_old = np.empty(n, dtype=np.int64)
    new_of_old[old_of_new[mask_real]] = np.nonzero(mask_real)[0]

    s_new = new_of_old[src]
    t_new = new_of_old[dst]
    pad_a = 0
    pad_b = ntot - 1 - LOW

    # --- final per-tile budgets (max over all 8 cores at local tile j)
    j_of_new = (np.arange(ntot) % s0) // P
    j_of_dst = j_of_new[t_new]
    maxna_j = np.zeros(tpc, dtype=np.int64)
    maxnb_j = np.zeros(tpc, dtype=np.int64)
    maxdeg_j = np.zeros(tpc, dtype=np.int64)
    np.maximum.at(maxna_j, j_of_dst, n_a[dst])
    np.maximum.at(maxnb_j, j_of_dst, n_b[dst])
    np.maximum.at(maxdeg_j, j_of_dst, deg[dst])
    ka_j = maxna_j
    kb_j = np.maximum(maxnb_j, maxdeg_j - ka_j)

    # --- final per-node split honoring the budgets
    jv = np.zeros(n, dtype=np.int64)
    jv[dst] = j_of_dst  # local tile j per dst node (only where deg>0 matters)
    lo_k = np.maximum(n_a, deg - kb_j[jv])
    hi_k = np.minimum(n_a + n_f, ka_j[jv])
    assert (lo_k <= hi_k).all()
    k_a = np.clip(ka_prov, lo_k, hi_k)
    x_to_a = k_a - n_a
    k_b = deg - k_a

    flex_idx = np.nonzero(flex)[0]
    o = np.argsort(dst[flex_idx], kind="stable")
    fi = flex_idx[o]
    t_sorted = dst[fi]
    grp_sizes = np.bincount(t_sorted, minlength=n)
    grp_start = np.concatenate([[0], np.cumsum(grp_sizes)])[:-1]
    pos_in = np.arange(len(fi)) - grp_start[t_sorted]
    in_a = forced_a.copy()
    in_a[fi[pos_in < x_to_a[t_sorted]]] = True

    # --- ELL buffers: per core, [P, S] int16 where S = sum_j (ka_j + kb_j);
    #     A-block of tile j occupies cols [offa[j], offa[j]+ka_j), then B-block.
    steps_j = ka_j + kb_j
    offa = np.zeros(tpc, dtype=np.int64)
    offa[1:] = np.cumsum(steps_j)[:-1]
    offb = offa + ka_j
    S = int(np.cumsum(steps_j)[-1]) if tpc > 0 else 0

    col_default = np.empty(S, dtype=np.int16)
    for j in range(tpc):
        col_default[offa[j]:offa[j] + ka_j[j]] = pad_a
        col_default[offb[j]:offb[j] + kb_j[j]] = pad_b
    ell = np.broadcast_to(col_default, (NCORES, P, S)).copy()

    def scatter_half(mask, values, off_arr):
        e = np.nonzero(mask)[0]
        t_e = t_new[e]
        o2 = np.argsort(t_e, kind="stable")
        e = e[o2]
        t_e = t_e[o2]
        gsz = np.bincount(t_e, minlength=ntot)
        gst = np.concatenate([[0], np.cumsum(gsz)])[:-1]
        k = np.arange(len(e)) - gst[t_e]
        c_e = t_e // s0
        p_e = t_e % P
        j_e = (t_e % s0) // P
        col = off_arr[j_e] + k
        ell[c_e, p_e, col] = values[e].astype(np.int16)

    scatter_half(in_a, s_new, offa)
    scatter_half(~in_a, s_new - LOW, offb)

    # --- wrap ELL blocks into dma_gather index layout:
    #     flat i = g*128 + p; stored at [i % 16, i // 16]; replicated to 128 par.
    call_list = []  # (j, 'A'/'B', col_off_in_idxbuf, K)
    idx_cols = 0
    for j in range(tpc):
        if ka_j[j] > 0:
            call_list.append((j, 0, idx_cols, int(ka_j[j])))
            idx_cols += int(ka_j[j]) * 8
        if kb_j[j] > 0:
            call_list.append((j, 1, idx_cols, int(kb_j[j])))
            idx_cols += int(kb_j[j]) * 8

    idx_all = np.zeros((NCORES, P, max(idx_cols, 8)), dtype=np.int16)
    for c in range(NCORES):
        for (j, half, coff, K) in call_list:
            src_off = offa[j] if half == 0 else offb[j]
            blk = ell[c, :, src_off:src_off + K]          # [P, K]
            flat = blk.T.reshape(-1)                      # i = g*128 + p
            w16 = flat.reshape(K * 8, 16).T               # [16, K*8]
            idx_all[c, :, coff:coff + K * 8] = np.tile(w16, (8, 1))

    # --- per-core dense inputs
    feat_new = np.zeros((ntot, din), dtype=np.float32)
    feat_new[new_of_old] = features.astype(np.float32)
    ns_new = np.zeros(ntot, dtype=np.float32)
    ns_new[new_of_old] = ns
    nd_new = np.zeros(ntot, dtype=np.float32)
    nd_new[new_of_old] = nd

    def per_core_scale(vec):
        return [
            np.ascontiguousarray(
                vec[c * s0:(c + 1) * s0].reshape(tpc, P).T
            ) for c in range(NCORES)
        ]

    xct = [
        np.ascontiguousarray(feat_new[c * s0:(c + 1) * s0].T)
        for c in range(NCORES)
    ]
    s0_scale = per_core_scale(ns_new)                 # table-1 scale
    s12_scale = per_core_scale(ns_new * nd_new)       # post-SpMM fused scale (l=1,2)
    s3_scale = per_core_scale(nd_new)                 # final scale
    snd_scale = per_core_scale(nd_new)                # bias path

    w3p = np.zeros((dhid, dcp), dtype=np.float32)
    w3p[:, :ncls] = W3.astype(np.float32)

    has_bias = bool(np.any(b1) or np.any(b2) or np.any(b3))
    brep1 = np.tile(np.asarray(b1, np.float32), (P, 1))
    brep2 = np.tile(np.asarray(b2, np.float32), (P, 1))
    brep3 = np.zeros((P, dcp), dtype=np.float32)
    brep3[:, :ncls] = np.asarray(b3, np.float32)

    meta = dict(
        n=n, din=din, dhid=dhid, ncls=ncls, dcp=dcp, tpc=tpc, s0=s0, ntot=ntot,
        LOW=LOW, HIGH=HIGH, call_list=call_list, idx_cols=int(max(idx_cols, 8)),
        ka_j=ka_j.tolist(), kb_j=kb_j.tolist(), has_bias=has_bias,
        old_of_new=old_of_new,
    )
    in_maps = []
    for c in range(NCORES):
        m = {
            "xct": xct[c],
            "w1": np.ascontiguousarray(W1, dtype=np.float32),
            "w2": np.ascontiguousarray(W2, dtype=np.float32),
            "w3p": w3p,
            "sc0": s0_scale[c],
            "sc12": s12_scale[c],
            "sc3": s3_scale[c],
            "idx": np.ascontiguousarray(idx_all[c]),
        }
        if has_bias:
            m["snd"] = snd_scale[c]
            m["brep1"] = brep1
            m["brep2"] = brep2
            m["brep3"] = brep3
        in_maps.append(m)
    return meta, in_maps


# ----------------------------------------------------------------------------
# Device program
# ----------------------------------------------------------------------------

def _build_program(meta, enable_asserts=False):
    import concourse.bacc as bacc
    import concourse.mybir as mybir
    import concourse.tile as tile
    from concourse.masks import make_identity

    f32 = mybir.dt.float32
    i16 = mybir.dt.int16
    Alu = mybir.AluOpType

    tpc, s0, ntot = meta["tpc"], meta["s0"], meta["ntot"]
    din, dhid, dcp = meta["din"], meta["dhid"], meta["dcp"]
    LOW, HIGH = meta["LOW"], meta["HIGH"]
    call_list = meta["call_list"]
    ka_j, kb_j = meta["ka_j"], meta["kb_j"]
    has_bias = meta["has_bias"]

    nc = bacc.Bacc(
        "TRN2", target_bir_lowering=False, debug=False,
        enable_asserts=enable_asserts, num_devices=NCORES,
    )

    xct = nc.dram_tensor("xct", [P, s0], f32, kind="ExternalInput")
    w1 = nc.dram_tensor("w1", [din, dhid], f32, kind="ExternalInput")
    w2 = nc.dram_tensor("w2", [dhid, dhid], f32, kind="ExternalInput")
    w3p = nc.dram_tensor("w3p", [dhid, dcp], f32, kind="ExternalInput")
    sc0 = nc.dram_tensor("sc0", [P, tpc], f32, kind="ExternalInput")
    sc12 = nc.dram_tensor("sc12", [P, tpc], f32, kind="ExternalInput")
    sc3 = nc.dram_tensor("sc3", [P, tpc], f32, kind="ExternalInput")
    idx = nc.dram_tensor("idx", [P, meta["idx_cols"]], i16, kind="ExternalInput")
    if has_bias:
        snd = nc.dram_tensor("snd", [P, tpc], f32, kind="ExternalInput")
        brep1 = nc.dram_tensor("brep1", [P, dhid], f32, kind="ExternalInput")
        brep2 = nc.dram_tensor("brep2", [P, dhid], f32, kind="ExternalInput")
        brep3 = nc.dram_tensor("brep3", [P, dcp], f32, kind="ExternalInput")
    outp = nc.dram_tensor("outp", [s0, dcp], f32, kind="ExternalOutput")

    rg = [list(range(NCORES))]

    with tile.TileContext(nc) as tc:
        with (
            tc.tile_pool(name="constp", bufs=1) as constp,
            tc.tile_pool(name="gatherp", bufs=3) as gatherp,
            tc.tile_pool(name="workp", bufs=3) as workp,
            tc.tile_pool(name="psumtp", bufs=2, space="PSUM") as psumtp,
            tc.tile_pool(name="psumzp", bufs=2, space="PSUM") as psumzp,
            tc.tile_pool(name="dramp", bufs=1, space="DRAM") as dramp,
        ):
            shared = "Shared" if os.environ.get("GCN_SHARED", "1") == "1" else "Local"
            z1 = dramp.tile([s0, dhid], f32)
            z2 = dramp.tile([s0, dhid], f32)
            z3 = dramp.tile([s0, dcp], f32)
            t1 = dramp.tile([ntot, dhid], f32, addr_space=shared)
            t2 = dramp.tile([ntot, dhid], f32, addr_space=shared)
            t3 = dramp.tile([ntot, dcp], f32, addr_space=shared)

            xct_sb = constp.tile([P, s0], f32)
            nc.sync.dma_start(out=xct_sb[:], in_=xct[:, :])
            w1_sb = constp.tile([P, dhid], f32)
            nc.sync.dma_start(out=w1_sb[:], in_=w1[:, :])
            w2_sb = constp.tile([P, dhid], f32)
            nc.sync.dma_start(out=w2_sb[:], in_=w2[:, :])
            w3_sb = constp.tile([P, dcp], f32)
            nc.sync.dma_start(out=w3_sb[:], in_=w3p[:, :])
            sc0_sb = constp.tile([P, tpc], f32)
            nc.sync.dma_start(out=sc0_sb[:], in_=sc0[:, :])
            sc12_sb = constp.tile([P, tpc], f32)
            nc.sync.dma_start(out=sc12_sb[:], in_=sc12[:, :])
            sc3_sb = constp.tile([P, tpc], f32)
            nc.sync.dma_start(out=sc3_sb[:], in_=sc3[:, :])
            idx_sb = constp.tile([P, meta["idx_cols"]], i16)
            nc.sync.dma_start(out=idx_sb[:], in_=idx[:, :])
            if has_bias:
                snd_sb = constp.tile([P, tpc], f32)
                nc.sync.dma_start(out=snd_sb[:], in_=snd[:, :])
                b1_sb = constp.tile([P, dhid], f32)
                nc.sync.dma_start(out=b1_sb[:], in_=brep1[:, :])
                b2_sb = constp.tile([P, dhid], f32)
                nc.sync.dma_start(out=b2_sb[:], in_=brep2[:, :])
                b3_sb = constp.tile([P, dcp], f32)
                nc.sync.dma_start(out=b3_sb[:], in_=brep3[:, :])
            ident = constp.tile([P, P], f32)
            make_identity(nc, ident[:])

            # ---- phase 0: z1 = ns * (X @ W1), node-major per tile
            for j in range(tpc):
                zp = psumzp.tile([P, dhid], f32, tag="zp")
                nc.tensor.matmul(
                    out=zp[:], lhsT=xct_sb[:, j * P:(j + 1) * P], rhs=w1_sb[:],
                    start=True, stop=True,
                )
                zt = workp.tile([P, dhid], f32, tag="zt")
                nc.vector.tensor_scalar(
                    out=zt[:], in0=zp[:], scalar1=sc0_sb[:, j:j + 1], scalar2=None,
                    op0=Alu.mult,
                )
                nc.sync.dma_start(out=z1[j * P:(j + 1) * P, :], in_=zt[:])

            nc.gpsimd.collective_compute(
                "AllGather", Alu.bypass, replica_groups=rg,
                ins=[z1[:].opt()], outs=[t1[:].opt()],
            )

            def spmm_layer(tbl, d_el, scale_sb, relu, wnext_sb, dnext, zout,
                           bias_sb):
                """One SpMM pass over all tiles; optionally produce the next
                table z-rows (wnext_sb is None for the final layer)."""
                for j in range(tpc):
                    ka, kb = ka_j[j], kb_j[j]
                    ktot = ka + kb
                    if ktot > 0:
                        gb = gatherp.tile([P, ktot, d_el], f32, tag="gb")
                        for (jj, half, coff, K) in call_list:
                            if jj != j:
                                continue
                            if half == 0:
                                o_lo = 0
                                tv = tbl[0:HIGH, :]
                            else:
                                o_lo = ka
                                tv = tbl[LOW:ntot, :]
                            # cap descriptors per call to fit the 1024-entry
                            # SWDGE ring: <= MAXK steps (MAXK*128 rows) each
                            for g0 in range(0, K, MAXK):
                                kc = min(MAXK, K - g0)
                                nc.gpsimd.dma_gather(
                                    out_ap=gb[:, o_lo + g0:o_lo + g0 + kc, :],
                                    in_ap=tv,
                                    idxs_ap=idx_sb[
                                        :, coff + g0 * 8:coff + (g0 + kc) * 8
                                    ],
                                    num_idxs=kc * P,
                                    num_idxs_reg=kc * P,
                                    elem_size=d_el,
                                )
                        # log-tree reduction over steps
                        k = ktot
                        while k > 1:
                            h = k // 2
                            nc.vector.tensor_tensor(
                                out=gb[:, 0:h, :], in0=gb[:, 0:h, :],
                                in1=gb[:, k - h:k, :], op=Alu.add,
                            )
                            k -= h
                        acc = gb[:, 0, :]
                    else:
                        gz = workp.tile([P, d_el], f32, tag="gz")
                        nc.vector.memset(gz[:], 0.0)
                        acc = gz[:]

                    ht = workp.tile([P, d_el], f32, tag="ht")
                    if not has_bias:
                        if relu:
                            nc.vector.tensor_scalar(
                                out=ht[:], in0=acc, scalar1=scale_sb[:, j:j + 1],
                                scalar2=0.0, op0=Alu.mult, op1=Alu.max,
                            )
                        else:
                            nc.vector.tensor_scalar(
                                out=ht[:], in0=acc, scalar1=scale_sb[:, j:j + 1],
                                scalar2=None, op0=Alu.mult,
                            )
                    else:
                        tmp = workp.tile([P, d_el], f32, tag="tmp")
                        nc.vector.tensor_scalar(
                            out=tmp[:], in0=acc, scalar1=snd_sb[:, j:j + 1],
                            scalar2=None, op0=Alu.mult,
                        )
                        nc.vector.tensor_tensor(
                            out=tmp[:], in0=tmp[:], in1=bias_sb[:], op=Alu.add,
                        )
                        if relu:
                            # relu then * ns  (scale_sb here must be ns)
                            nc.vector.tensor_scalar(
                                out=ht[:], in0=tmp[:], scalar1=0.0,
                                scalar2=scale_sb[:, j:j + 1],
                                op0=Alu.max, op1=Alu.mult,
                            )
                        else:
                            nc.vector.tensor_copy(out=ht[:], in_=tmp[:])

                    if wnext_sb is None:
                        nc.sync.dma_start(
                            out=outp[j * P:(j + 1) * P, :], in_=ht[:],
                        )
                    else:
                        tp = psumtp.tile([P, P], f32, tag="tp")
                        nc.tensor.transpose(out=tp[:], in_=ht[:], identity=ident[:])
                        htT = workp.tile([P, P], f32, tag="htT")
                        nc.vector.tensor_copy(out=htT[:], in_=tp[:])
                        zp = psumzp.tile([P, dnext], f32, tag="zp")
                        nc.tensor.matmul(
                            out=zp[:], lhsT=htT[:], rhs=wnext_sb[:],
                            start=True, stop=True,
                        )
                        zt = workp.tile([P, dnext], f32, tag="zt")
                        nc.vector.tensor_copy(out=zt[:], in_=zp[:])
                        nc.sync.dma_start(
                            out=zout[j * P:(j + 1) * P, :], in_=zt[:],
                        )

            # layer 1: gather t1, produce z2 -> t2
            # (bias path: post-relu multiplier is ns alone = sc0; fused path
            #  uses ns*nd = sc12 inside the relu)
            sc_mid = sc0_sb if has_bias else sc12_sb
            spmm_layer(t1, dhid, sc_mid, True, w2_sb, dhid, z2,
                       b1_sb if has_bias else None)
            nc.gpsimd.collective_compute(
                "AllGather", Alu.bypass, replica_groups=rg,
                ins=[z2[:].opt()], outs=[t2[:].opt()],
            )
            # layer 2: gather t2, produce z3 -> t3
            spmm_layer(t2, dhid, sc_mid, True, w3_sb, dcp, z3,
                       b2_sb if has_bias else None)
            nc.gpsimd.collective_compute(
                "AllGather", Alu.bypass, replica_groups=rg,
                ins=[z3[:].opt()], outs=[t3[:].opt()],
            )
            # layer 3: gather t3, final scale, write out
            spmm_layer(t3, dcp, sc3_sb, False, None, None, None,
                       b3_sb if has_bias else None)

    nc.compile()
    return nc


# ----------------------------------------------------------------------------
# Entry point
# ----------------------------------------------------------------------------

_CACHE = {}


def _graph_key(edge_index, shapes):
    e = np.asarray(edge_index)
    return (e.shape, hash(e.tobytes()), shapes)


def run(inputs, trace=False, trace_cores=None):
    """Full pipeline; returns (output, BassKernelResults)."""
    features = np.asarray(inputs["features"], dtype=np.float32)
    edge_index = np.asarray(inputs["edge_index"])
    W1, b1 = np.asarray(inputs["W1"]), np.asarray(inputs["b1"])
    W2, b2 = np.asarray(inputs["W2"]), np.asarray(inputs["b2"])
    W3, b3 = np.asarray(inputs["W3"]), np.asarray(inputs["b3"])

    meta, in_maps = _preprocess(features, edge_index, W1, b1, W2, b2, W3, b3)
    key = _graph_key(edge_index, (features.shape, meta["has_bias"]))
    if key not in _CACHE:
        _CACHE[key] = _build_program(meta)
    nc = _CACHE[key]

    import concourse.bass_utils as bass_utils

    res = bass_utils.run_bass_kernel_spmd(
        nc, in_maps, core_ids=list(range(NCORES)),
        trace=trace, trace_cores=trace_cores,
    )
    return _assemble(meta, [r["outp"] for r in res.results]), res


def kernel(**inputs):
    return run(inputs)[0]


def _assemble(meta, outs):
    n, ncls, s0 = meta["n"], meta["ncls"], meta["s0"]
    old_of_new = meta["old_of_new"]
    full = np.concatenate(outs, axis=0)  # [ntot, dcp]
    result = np.empty((n, ncls), dtype=np.float32)
    mask = old_of_new >= 0
    result[old_of_new[mask]] = full[mask][:, :ncls]
    return result


# revision 12
# speedup vs baseline: 1.2625x; 1.2625x over previous
"""GCN (3-layer GraphConv, norm='both') on 8 Trainium2 NeuronCores.

Self-contained: takes FULL inputs, returns FULL output [N, n_classes].

Strategy
--------
Math: per layer  out = nd * (A^T (ns * (x @ W))) + b,  nd=deg_in^-1/2, ns=deg_out^-1/2
(W pushed in front of the SpMM by associativity; for layer 3 this shrinks the
gather width from 128 to 40->64 floats).

- Nodes are sorted by in-degree (desc) and chopped into 128-row tiles; tiles are
  dealt round-robin to the 8 cores so every core's j-th tile has near-identical
  degree profile (SPMD: one NEFF, per-core index data).
- Per layer, each core computes the table rows Z = ns*(H@W) for its own nodes
  (node-major [6272, d]) and an AllGather materializes the full table
  [50176, d] in every core's DRAM (addr_space="Shared").
- SpMM = ELL gather-accumulate: for each dst tile, dma_gather pulls the source
  rows (one row per (slot, step)), then a log-tree of wide DVE adds reduces the
  steps. dma_gather indices are int16, so the table is addressed through two
  overlapping views A=[0,32768) and B=[NTOT-32768,NTOT); each node's in-edges
  are split between the views (flexible middle zone balances the split).
  Padding slots gather a guaranteed-zero table row (virtual node rows).
- relu(nd*agg)*ns == relu(nd*ns*agg) for zero bias -> single fused dual-op
  tensor_scalar per tile. (Non-zero bias gets an explicit slower path.)
- Next-layer table tile: PE transpose (identity) -> matmul with W -> DMA out.
"""

import math
import os

import numpy as np

P = 128
NCORES = 8
MAXK = int(os.environ.get("GCN_MAXK", "8"))  # max gather steps per dma_gather
SINGLE_PACKET = os.environ.get("GCN_SP", "1") == "1"
NSWQ = int(os.environ.get("GCN_NSWQ", "1"))  # SWDGE queues (round-robin)


# ----------------------------------------------------------------------------
# Host-side preprocessing
# ----------------------------------------------------------------------------

def _preprocess(features, edge_index, W1, b1, W2, b2, W3, b3):
    n, din = features.shape
    dhid = W2.shape[0]
    ncls = W3.shape[1]
    assert din == P and dhid == P, "kernel assumes 128-wide features"
    dcp = ((ncls + 63) // 64) * 64  # pad classes to 64 (256B rows for dma_gather)

    src = np.asarray(edge_index[0], dtype=np.int64)
    dst = np.asarray(edge_index[1], dtype=np.int64)

    deg_out = np.bincount(src, minlength=n).astype(np.float32)
    deg_in = np.bincount(dst, minlength=n).astype(np.float32)
    ns = np.maximum(deg_out, 1.0) ** -0.5
    nd = np.maximum(deg_in, 1.0) ** -0.5

    # --- layout sizes; table views (int16 index limit 32767)
    tpc = math.ceil((n + 1) / (P * NCORES))       # tiles per core
    s0 = tpc * P                                   # slots per core
    ntot = NCORES * s0
    nv = ntot - n                                  # virtual (zero) rows
    assert nv >= 1
    HIGH = min(32768, ntot)   # view A = [0, HIGH)
    LOW = ntot - HIGH         # view B = [LOW, ntot)

    # Table zones decide edge forcing: sources with table id in
    #   [0, LOW)      -> only reachable via view A  ("forced A")
    #   [LOW, HIGH)   -> reachable via both views    ("flex")
    #   [HIGH, ntot)  -> only view B                 ("forced B")
    zone_lo = np.array([0, LOW, HIGH])
    zone_hi = np.array([LOW, HIGH, ntot])
    zone_size = zone_hi - zone_lo
    # reserved virtual table ids: 0 (view-A zero row) and the tail
    # [ntot-nv+1, ntot) (so id ntot-1 is the view-B zero row)
    resv = np.zeros(3, dtype=np.int64)
    z_of_id0 = 0 if LOW > 0 else 1
    resv[z_of_id0] += 1
    tail_lo = ntot - (nv - 1)
    for z in range(3):
        resv[z] += max(0, min(zone_hi[z], ntot) - max(zone_lo[z], tail_lo))
    real_cap = zone_size - resv
    assert real_cap.sum() == n and (real_cap >= 0).all()

    # --- assign real nodes to zones: degree-sorted, dealt proportionally so
    #     every zone sees the same degree profile
    order = np.argsort(-deg_in, kind="stable")
    zone_of_old = np.empty(n, dtype=np.int8)
    cnt = np.zeros(3, dtype=np.int64)
    err = np.zeros(3)
    frac = real_cap / max(n, 1)
    for idx in order:
        err += frac
        z = int(np.argmax(np.where(cnt < real_cap, err, -np.inf)))
        err[z] -= 1.0
        cnt[z] += 1
        zone_of_old[idx] = z

    # --- forcing + provisional balanced split per dst node
    sz = zone_of_old[src]
    forced_a = sz == 0
    forced_b = sz == 2
    flex = sz == 1
    n_a = np.bincount(dst[forced_a], minlength=n)
    n_b = np.bincount(dst[forced_b], minlength=n)
    n_f = np.bincount(dst[flex], minlength=n)
    deg = n_a + n_b + n_f
    x_prov = np.clip((n_b - n_a + n_f + 1) // 2, 0, n_f)
    ka_prov = n_a + x_prov
    kb_prov = n_b + (n_f - x_prov)

    # --- group profile-similar nodes into tiles; place tiles so that the
    #     8 cores' j-th tiles have matching profiles (SPMD-uniform K)
    queues = []
    for z in range(3):
        ids_z = np.nonzero(zone_of_old == z)[0]
        o = np.lexsort((-deg[ids_z], -kb_prov[ids_z], -ka_prov[ids_z]))
        q = ids_z[o].tolist()
        if z == z_of_id0:
            q = [-1] + q
        # tail virtuals for this zone
        extra = int(resv[z] - (1 if z == z_of_id0 else 0))
        q = q + [-1] * extra
        queues.append(q)
    cur = [0, 0, 0]
    old_of_new = np.full(ntot, -1, dtype=np.int64)
    for j in range(tpc):
        for c in range(NCORES):
            base = c * s0 + j * P
            z = 0 if base < LOW else (1 if base < HIGH else 2)
            chunk = queues[z][cur[z]:cur[z] + P]
            cur[z] += P
            old_of_new[base:base + P] = chunk
    assert all(cur[z] == len(queues[z]) for z in range(3))
    assert old_of_new[0] == -1 and old_of_new[ntot - 1] == -1
    mask_real = old_of_new >= 0
    new_of_old = np.empty(n, dtype=np.int64)
    new_of_old[old_of_new[mask_real]] = np.nonzero(mask_real)[0]

    s_new = new_of_old[src]
    t_new = new_of_old[dst]
    pad_a = 0
    pad_b = ntot - 1 - LOW

    # --- final per-tile budgets (max over all 8 cores at local tile j)
    j_of_new = (np.arange(ntot) % s0) // P
    j_of_dst = j_of_new[t_new]
    maxna_j = np.zeros(tpc, dtype=np.int64)
    maxnb_j = np.zeros(tpc, dtype=np.int64)
    maxdeg_j = np.zeros(tpc, dtype=np.int64)
    np.maximum.at(maxna_j, j_of_dst, n_a[dst])
    np.maximum.at(maxnb_j, j_of_dst, n_b[dst])
    np.maximum.at(maxdeg_j, j_of_dst, deg[dst])
    ka_j = maxna_j
    kb_j = np.maximum(maxnb_j, maxdeg_j - ka_j)

    # --- final per-node split honoring the budgets
    jv = np.zeros(n, dtype=np.int64)
    jv[dst] = j_of_dst  # local tile j per dst node (only where deg>0 matters)
    lo_k = np.maximum(n_a, deg - kb_j[jv])
    hi_k = np.minimum(n_a + n_f, ka_j[jv])
    assert (lo_k <= hi_k).all()
    k_a = np.clip(ka_prov, lo_k, hi_k)
    x_to_a = k_a - n_a
    k_b = deg - k_a

    flex_idx = np.nonzero(flex)[0]
    o = np.argsort(dst[flex_idx], kind="stable")
    fi = flex_idx[o]
    t_sorted = dst[fi]
    grp_sizes = np.bincount(t_sorted, minlength=n)
    grp_start = np.concatenate([[0], np.cumsum(grp_sizes)])[:-1]
    pos_in = np.arange(len(fi)) - grp_start[t_sorted]
    in_a = forced_a.copy()
    in_a[fi[pos_in < x_to_a[t_sorted]]] = True

    # --- ELL buffers: per core, [P, S] int16 where S = sum_j (ka_j + kb_j);
    #     A-block of tile j occupies cols [offa[j], offa[j]+ka_j), then B-block.
    steps_j = ka_j + kb_j
    offa = np.zeros(tpc, dtype=np.int64)
    offa[1:] = np.cumsum(steps_j)[:-1]
    offb = offa + ka_j
    S = int(np.cumsum(steps_j)[-1]) if tpc > 0 else 0

    col_default = np.empty(S, dtype=np.int16)
    for j in range(tpc):
        col_default[offa[j]:offa[j] + ka_j[j]] = pad_a
        col_default[offb[j]:offb[j] + kb_j[j]] = pad_b
    ell = np.broadcast_to(col_default, (NCORES, P, S)).copy()

    def scatter_half(mask, values, off_arr):
        e = np.nonzero(mask)[0]
        t_e = t_new[e]
        o2 = np.argsort(t_e, kind="stable")
        e = e[o2]
        t_e = t_e[o2]
        gsz = np.bincount(t_e, minlength=ntot)
        gst = np.concatenate([[0], np.cumsum(gsz)])[:-1]
        k = np.arange(len(e)) - gst[t_e]
        c_e = t_e // s0
        p_e = t_e % P
        j_e = (t_e % s0) // P
        col = off_arr[j_e] + k
        ell[c_e, p_e, col] = values[e].astype(np.int16)

    scatter_half(in_a, s_new, offa)
    scatter_half(~in_a, s_new - LOW, offb)

    # --- wrap ELL blocks into dma_gather index layout:
    #     flat i = g*128 + p; stored at [i % 16, i // 16]; replicated to 128 par.
    call_list = []  # (j, 'A'/'B', col_off_in_idxbuf, K)
    idx_cols = 0
    for j in range(tpc):
        if ka_j[j] > 0:
            call_list.append((j, 0, idx_cols, int(ka_j[j])))
            idx_cols += int(ka_j[j]) * 8
        if kb_j[j] > 0:
            call_list.append((j, 1, idx_cols, int(kb_j[j])))
            idx_cols += int(kb_j[j]) * 8

    idx_all = np.zeros((NCORES, P, max(idx_cols, 8)), dtype=np.int16)
    for c in range(NCORES):
        for (j, half, coff, K) in call_list:
            src_off = offa[j] if half == 0 else offb[j]
            blk = ell[c, :, src_off:src_off + K]          # [P, K]
            flat = blk.T.reshape(-1)                      # i = g*128 + p
            w16 = flat.reshape(K * 8, 16).T               # [16, K*8]
            idx_all[c, :, coff:coff + K * 8] = np.tile(w16, (8, 1))

    # --- per-core dense inputs
    feat_new = np.zeros((ntot, din), dtype=np.float32)
    feat_new[new_of_old] = features.astype(np.float32)
    ns_new = np.zeros(ntot, dtype=np.float32)
    ns_new[new_of_old] = ns
    nd_new = np.zeros(ntot, dtype=np.float32)
    nd_new[new_of_old] = nd

    def per_core_scale(vec):
        return [
            np.ascontiguousarray(
                vec[c * s0:(c + 1) * s0].reshape(tpc, P).T
            ) for c in range(NCORES)
        ]

    xct = [
        np.ascontiguousarray(feat_new[c * s0:(c + 1) * s0].T)
        for c in range(NCORES)
    ]
    s0_scale = per_core_scale(ns_new)                 # table-1 scale
    s12_scale = per_core_scale(ns_new * nd_new)       # post-SpMM fused scale (l=1,2)
    s3_scale = per_core_scale(nd_new)                 # final scale
    snd_scale = per_core_scale(nd_new)                # bias path

    w3p = np.zeros((dhid, dcp), dtype=np.float32)
    w3p[:, :ncls] = W3.astype(np.float32)

    has_bias = bool(np.any(b1) or np.any(b2) or np.any(b3))
    brep1 = np.tile(np.asarray(b1, np.float32), (P, 1))
    brep2 = np.tile(np.asarray(b2, np.float32), (P, 1))
    brep3 = np.zeros((P, dcp), dtype=np.float32)
    brep3[:, :ncls] = np.asarray(b3, np.float32)

    meta = dict(
        n=n, din=din, dhid=dhid, ncls=ncls, dcp=dcp, tpc=tpc, s0=s0, ntot=ntot,
        LOW=LOW, HIGH=HIGH, call_list=call_list, idx_cols=int(max(idx_cols, 8)),
        ka_j=ka_j.tolist(), kb_j=kb_j.tolist(), has_bias=has_bias,
        old_of_new=old_of_new,
    )
    in_maps = []
    for c in range(NCORES):
        m = {
            "xct": xct[c],
            "w1": np.ascontiguousarray(W1, dtype=np.float32),
            "w2": np.ascontiguousarray(W2, dtype=np.float32),
            "w3p": w3p,
            "sc0": s0_scale[c],
            "sc12": s12_scale[c],
            "sc3": s3_scale[c],
            "idx": np.ascontiguousarray(idx_all[c]),
        }
        if has_bias:
            m["snd"] = snd_scale[c]
            m["brep1"] = brep1
            m["brep2"] = brep2
            m["brep3"] = brep3
        in_maps.append(m)
    return meta, in_maps


# ----------------------------------------------------------------------------
# Device program
# ----------------------------------------------------------------------------

def _build_program(meta, enable_asserts=False):
    import concourse.bacc as bacc
    import concourse.mybir as mybir
    import concourse.tile as tile
    from concourse.masks import make_identity

    f32 = mybir.dt.float32
    i16 = mybir.dt.int16
    Alu = mybir.AluOpType

    tpc, s0, ntot = meta["tpc"], meta["s0"], meta["ntot"]
    din, dhid, dcp = meta["din"], meta["dhid"], meta["dcp"]
    LOW, HIGH = meta["LOW"], meta["HIGH"]
    call_list = meta["call_list"]
    ka_j, kb_j = meta["ka_j"], meta["kb_j"]
    has_bias = meta["has_bias"]

    nc = bacc.Bacc(
        "TRN2", target_bir_lowering=False, debug=False,
        enable_asserts=enable_asserts, num_devices=NCORES,
        num_swdge_queues=NSWQ,
    )
    qctr = [0]

    xct = nc.dram_tensor("xct", [P, s0], f32, kind="ExternalInput")
    w1 = nc.dram_tensor("w1", [din, dhid], f32, kind="ExternalInput")
    w2 = nc.dram_tensor("w2", [dhid, dhid], f32, kind="ExternalInput")
    w3p = nc.dram_tensor("w3p", [dhid, dcp], f32, kind="ExternalInput")
    sc0 = nc.dram_tensor("sc0", [P, tpc], f32, kind="ExternalInput")
    sc12 = nc.dram_tensor("sc12", [P, tpc], f32, kind="ExternalInput")
    sc3 = nc.dram_tensor("sc3", [P, tpc], f32, kind="ExternalInput")
    idx = nc.dram_tensor("idx", [P, meta["idx_cols"]], i16, kind="ExternalInput")
    if has_bias:
        snd = nc.dram_tensor("snd", [P, tpc], f32, kind="ExternalInput")
        brep1 = nc.dram_tensor("brep1", [P, dhid], f32, kind="ExternalInput")
        brep2 = nc.dram_tensor("brep2", [P, dhid], f32, kind="ExternalInput")
        brep3 = nc.dram_tensor("brep3", [P, dcp], f32, kind="ExternalInput")
    outp = nc.dram_tensor("outp", [s0, dcp], f32, kind="ExternalOutput")

    rg = [list(range(NCORES))]

    with tile.TileContext(nc) as tc:
        with (
            tc.tile_pool(name="constp", bufs=1) as constp,
            tc.tile_pool(name="gatherp", bufs=3) as gatherp,
            tc.tile_pool(name="workp", bufs=3) as workp,
            tc.tile_pool(name="psumtp", bufs=2, space="PSUM") as psumtp,
            tc.tile_pool(name="psumzp", bufs=2, space="PSUM") as psumzp,
            tc.tile_pool(name="dramp", bufs=1, space="DRAM") as dramp,
        ):
            shared = "Shared" if os.environ.get("GCN_SHARED", "1") == "1" else "Local"
            z1 = dramp.tile([s0, dhid], f32)
            z2 = dramp.tile([s0, dhid], f32)
            z3 = dramp.tile([s0, dcp], f32)
            t1 = dramp.tile([ntot, dhid], f32, addr_space=shared)
            t2 = dramp.tile([ntot, dhid], f32, addr_space=shared)
            t3 = dramp.tile([ntot, dcp], f32, addr_space=shared)

            xct_sb = constp.tile([P, s0], f32)
            nc.sync.dma_start(out=xct_sb[:], in_=xct[:, :])
            w1_sb = constp.tile([P, dhid], f32)
            nc.sync.dma_start(out=w1_sb[:], in_=w1[:, :])
            w2_sb = constp.tile([P, dhid], f32)
            nc.sync.dma_start(out=w2_sb[:], in_=w2[:, :])
            w3_sb = constp.tile([P, dcp], f32)
            nc.sync.dma_start(out=w3_sb[:], in_=w3p[:, :])
            sc0_sb = constp.tile([P, tpc], f32)
            nc.sync.dma_start(out=sc0_sb[:], in_=sc0[:, :])
            sc12_sb = constp.tile([P, tpc], f32)
            nc.sync.dma_start(out=sc12_sb[:], in_=sc12[:, :])
            sc3_sb = constp.tile([P, tpc], f32)
            nc.sync.dma_start(out=sc3_sb[:], in_=sc3[:, :])
            idx_sb = constp.tile([P, meta["idx_cols"]], i16)
            nc.sync.dma_start(out=idx_sb[:], in_=idx[:, :])
            if has_bias:
                snd_sb = constp.tile([P, tpc], f32)
                nc.sync.dma_start(out=snd_sb[:], in_=snd[:, :])
                b1_sb = constp.tile([P, dhid], f32)
                nc.sync.dma_start(out=b1_sb[:], in_=brep1[:, :])
                b2_sb = constp.tile([P, dhid], f32)
                nc.sync.dma_start(out=b2_sb[:], in_=brep2[:, :])
                b3_sb = constp.tile([P, dcp], f32)
                nc.sync.dma_start(out=b3_sb[:], in_=brep3[:, :])
            ident = constp.tile([P, P], f32)
            make_identity(nc, ident[:])

            # ---- phase 0: z1 = ns * (X @ W1), node-major per tile
            for j in range(tpc):
                zp = psumzp.tile([P, dhid], f32, tag="zp")
                nc.tensor.matmul(
                    out=zp[:], lhsT=xct_sb[:, j * P:(j + 1) * P], rhs=w1_sb[:],
                    start=True, stop=True,
                )
                zt = workp.tile([P, dhid], f32, tag="zt")
                nc.vector.tensor_scalar(
                    out=zt[:], in0=zp[:], scalar1=sc0_sb[:, j:j + 1], scalar2=None,
                    op0=Alu.mult,
                )
                nc.sync.dma_start(out=z1[j * P:(j + 1) * P, :], in_=zt[:])

            nc.gpsimd.collective_compute(
                "AllGather", Alu.bypass, replica_groups=rg,
                ins=[z1[:].opt()], outs=[t1[:].opt()],
            )

            def spmm_layer(tbl, d_el, scale_sb, relu, wnext_sb, dnext, zout,
                           bias_sb):
                """One SpMM pass over all tiles; optionally produce the next
                table z-rows (wnext_sb is None for the final layer)."""
                for j in range(tpc):
                    ka, kb = ka_j[j], kb_j[j]
                    ktot = ka + kb
                    if ktot > 0:
                        gb = gatherp.tile([P, ktot, d_el], f32, tag="gb")
                        for (jj, half, coff, K) in call_list:
                            if jj != j:
                                continue
                            if half == 0:
                                o_lo = 0
                                tv = tbl[0:HIGH, :]
                            else:
                                o_lo = ka
                                tv = tbl[LOW:ntot, :]
                            # cap descriptors per call to fit the 1024-entry
                            # SWDGE ring: <= MAXK steps (MAXK*128 rows) each
                            for g0 in range(0, K, MAXK):
                                kc = min(MAXK, K - g0)
                                nc.gpsimd.dma_gather(
                                    out_ap=gb[:, o_lo + g0:o_lo + g0 + kc, :],
                                    in_ap=tv,
                                    idxs_ap=idx_sb[
                                        :, coff + g0 * 8:coff + (g0 + kc) * 8
                                    ],
                                    num_idxs=kc * P,
                                    num_idxs_reg=kc * P,
                                    elem_size=d_el,
                                    single_packet=SINGLE_PACKET,
                                    queue_num=qctr[0] % NSWQ,
                                )
                                qctr[0] += 1
                        # log-tree reduction over steps
                        k = ktot
                        while k > 1:
                            h = k // 2
                            nc.vector.tensor_tensor(
                                out=gb[:, 0:h, :], in0=gb[:, 0:h, :],
                                in1=gb[:, k - h:k, :], op=Alu.add,
                            )
                            k -= h
                        acc = gb[:, 0, :]
                    else:
                        gz = workp.tile([P, d_el], f32, tag="gz")
                        nc.vector.memset(gz[:], 0.0)
                        acc = gz[:]

                    ht = workp.tile([P, d_el], f32, tag="ht")
                    if not has_bias:
                        if relu:
                            nc.vector.tensor_scalar(
                                out=ht[:], in0=acc, scalar1=scale_sb[:, j:j + 1],
                                scalar2=0.0, op0=Alu.mult, op1=Alu.max,
                            )
                        else:
                            nc.vector.tensor_scalar(
                                out=ht[:], in0=acc, scalar1=scale_sb[:, j:j + 1],
                                scalar2=None, op0=Alu.mult,
                            )
                    else:
                        tmp = workp.tile([P, d_el], f32, tag="tmp")
                        nc.vector.tensor_scalar(
                            out=tmp[:], in0=acc, scalar1=snd_sb[:, j:j + 1],
                            scalar2=None, op0=Alu.mult,
                        )
                        nc.vector.tensor_tensor(
                            out=tmp[:], in0=tmp[:], in1=bias_sb[:], op=Alu.add,
                        )
                        if relu:
                            # relu then * ns  (scale_sb here must be ns)
                            nc.vector.tensor_scalar(
                                out=ht[:], in0=tmp[:], scalar1=0.0,
                                scalar2=scale_sb[:, j:j + 1],
                                op0=Alu.max, op1=Alu.mult,
                            )
                        else:
                            nc.vector.tensor_copy(out=ht[:], in_=tmp[:])

                    if wnext_sb is None:
                        nc.sync.dma_start(
                            out=outp[j * P:(j + 1) * P, :], in_=ht[:],
                        )
                    else:
                        tp = psumtp.tile([P, P], f32, tag="tp")
                        nc.tensor.transpose(out=tp[:], in_=ht[:], identity=ident[:])
                        htT = workp.tile([P, P], f32, tag="htT")
                        nc.vector.tensor_copy(out=htT[:], in_=tp[:])
                        zp = psumzp.tile([P, dnext], f32, tag="zp")
                        nc.tensor.matmul(
                            out=zp[:], lhsT=htT[:], rhs=wnext_sb[:],
                            start=True, stop=True,
                        )
                        zt = workp.tile([P, dnext], f32, tag="zt")
                        nc.vector.tensor_copy(out=zt[:], in_=zp[:])
                        nc.sync.dma_start(
                            out=zout[j * P:(j + 1) * P, :], in_=zt[:],
                        )

            # layer 1: gather t1, produce z2 -> t2
            # (bias path: post-relu multiplier is ns alone = sc0; fused path
            #  uses ns*nd = sc12 inside the relu)
            sc_mid = sc0_sb if has_bias else sc12_sb
            spmm_layer(t1, dhid, sc_mid, True, w2_sb, dhid, z2,
                       b1_sb if has_bias else None)
            nc.gpsimd.collective_compute(
                "AllGather", Alu.bypass, replica_groups=rg,
                ins=[z2[:].opt()], outs=[t2[:].opt()],
            )
            # layer 2: gather t2, produce z3 -> t3
            spmm_layer(t2, dhid, sc_mid, True, w3_sb, dcp, z3,
                       b2_sb if has_bias else None)
            nc.gpsimd.collective_compute(
                "AllGather", Alu.bypass, replica_groups=rg,
                ins=[z3[:].opt()], outs=[t3[:].opt()],
            )
            # layer 3: gather t3, final scale, write out
            spmm_layer(t3, dcp, sc3_sb, False, None, None, None,
                       b3_sb if has_bias else None)

    nc.compile()
    return nc


# ----------------------------------------------------------------------------
# Entry point
# ----------------------------------------------------------------------------

_CACHE = {}


def _graph_key(edge_index, shapes):
    e = np.asarray(edge_index)
    return (e.shape, hash(e.tobytes()), shapes)


def run(inputs, trace=False, trace_cores=None):
    """Full pipeline; returns (output, BassKernelResults)."""
    features = np.asarray(inputs["features"], dtype=np.float32)
    edge_index = np.asarray(inputs["edge_index"])
    W1, b1 = np.asarray(inputs["W1"]), np.asarray(inputs["b1"])
    W2, b2 = np.asarray(inputs["W2"]), np.asarray(inputs["b2"])
    W3, b3 = np.asarray(inputs["W3"]), np.asarray(inputs["b3"])

    meta, in_maps = _preprocess(features, edge_index, W1, b1, W2, b2, W3, b3)
    key = _graph_key(edge_index, (features.shape, meta["has_bias"]))
    if key not in _CACHE:
        _CACHE[key] = _build_program(meta)
    nc = _CACHE[key]

    import concourse.bass_utils as bass_utils

    res = bass_utils.run_bass_kernel_spmd(
        nc, in_maps, core_ids=list(range(NCORES)),
        trace=trace, trace_cores=trace_cores,
    )
    return _assemble(meta, [r["outp"] for r in res.results]), res


def kernel(**inputs):
    return run(inputs)[0]


def _assemble(meta, outs):
    n, ncls, s0 = meta["n"], meta["ncls"], meta["s0"]
    old_of_new = meta["old_of_new"]
    full = np.concatenate(outs, axis=0)  # [ntot, dcp]
    result = np.empty((n, ncls), dtype=np.float32)
    mask = old_of_new >= 0
    result[old_of_new[mask]] = full[mask][:, :ncls]
    return result
